# revision 27
# baseline (speedup 1.0000x reference)
"""Causal dilated conv1d (K=3, dilation=2, N=128 channels) on Trainium2.

out[b,t,i] = sum_{j,k} x[b, t-2k, j] * weight[i,j,k] + bias[i]

Strategy (8-core SPMD, pure data parallel over batch; bf16 datapath):
  - each core handles 4 of the 32 batch rows; weight/bias replicated.
  - host interleaves the core's 4 rows with a 16-row zero head:
    x4[tt, q, j] = x[b_q, tt-16, j]  (zeros for tt<16).  DMA xbar
    transposes load each slab directly as [128(j), q, t] strips in SBUF
    (transposed row r = q*128+j lands partition-first), so one transfer
    feeds multiple rows' strips, the PE does no transpose-in, and the
    zero head doubles as causal pad (no edge special-casing).  Each
    slab's transpose is split by row-pair across the two HWDGE queues
    (sync: q2/q3, act: q0/q1) so the two xbar streams run concurrently.
  - per-slab strip tiles (fresh pool tile per slab) keep the slab
    transposes free of false WAR deps: the tile framework tracks
    hazards at tile granularity, not byte ranges.
  - taps: 3 accumulated bf16 matmuls per 512-wide PSUM window, moving
    operand = strip shifted by 2k columns.  This is the ONLY PE work.
  - bias is added during the PSUM->SBUF copy, alternating between the
    DVE (tensor_scalar_add) and ACT (scalar.add) so neither engine is
    the bottleneck and ACT has room to host its xbar half-transposes.
  - the result stays in [i, t] layout: it is stored transposed to DRAM
    ([Q, 128, T], 128 contiguous descriptors per store) and the HOST
    restores [Q, T, 128] during the gather.  This removes the entire
    on-device transpose-out (PE identity matmuls + PSUM + DVE copies).
  - variable slab sizes (1024, 2048x3, 1024): small first slab starts
    the PE earlier, small last slab drains the tail faster.
  - output is bf16; host upconverts to fp32.
"""

import threading

import numpy as np
import ml_dtypes

import concourse.bass as bass  # noqa: F401  (bass types used via bacc/tile)
import concourse.mybir as mybir
import concourse.tile as tile
from concourse import bacc
from concourse.bass_utils import run_bass_kernel_spmd

P = 128
KTAPS = 3
DIL = 2
NCORES = 8
B_FULL, T_FULL = 32, 8192
B_CORE = B_FULL // NCORES  # 4
HEAD = 16  # zero rows prepended on host (causal pad + keeps slabs aligned)

FP32 = mybir.dt.float32
BF16 = mybir.dt.bfloat16


def build(T=T_FULL, slabs=(1024, 2048, 2048, 2048, 1024)):
    """Build the per-core Bass module. Same NEFF runs SPMD on all 8 cores."""
    assert sum(slabs) == T
    nc = bacc.Bacc(
        "TRN2",
        target_bir_lowering=False,
        debug=False,
        enable_asserts=False,
        num_devices=NCORES,
    )
    Q = B_CORE  # interleaved batch rows per core
    x_d = nc.dram_tensor("x", [HEAD + T, Q, P], BF16, kind="ExternalInput")
    w_d = nc.dram_tensor("w", [P, KTAPS * P], BF16, kind="ExternalInput")
    b_d = nc.dram_tensor("b", [P, 1], FP32, kind="ExternalInput")
    # output stays transposed on device: o[q, i, t]; host restores [q, t, i]
    o_d = nc.dram_tensor("o", [Q, P, T], BF16, kind="ExternalOutput")

    x_ap, o_ap = x_d.ap(), o_d.ap()
    SW = 512  # tap-matmul moving width (1 PSUM bank of fp32)
    n_slabs = len(slabs)
    starts = [sum(slabs[:i]) for i in range(n_slabs)]
    maxc = max(slabs)

    with tile.TileContext(nc) as tc:
        with (
            tc.tile_pool(name="const", bufs=1) as cp,
            tc.tile_pool(name="strip", bufs=2 * n_slabs) as sp,
            tc.tile_pool(name="oT", bufs=6) as otp,
            tc.tile_pool(name="pacc", bufs=6, space="PSUM") as paccp,
        ):
            w_sb = cp.tile([P, KTAPS * P], BF16)
            nc.sync.dma_start(w_sb[:], w_d.ap())
            bias_sb = cp.tile([P, 1], FP32)
            nc.sync.dma_start(bias_sb[:], b_d.ap())

            # one strip tile per HALF-slab (rows q0/q1 and q2/q3): all halves
            # are independent tiles, so the xbar transposes stream
            # back-to-back on the sync queue with no false WAR deps, and
            # taps for a half start as soon as that half has landed.
            # (Two CONCURRENT xbar streams on sync+act corrupt data — the
            # xbar is a single shared unit — so everything stays on sync.)
            strips = [
                [
                    sp.tile(
                        [P, 2 * (maxc + HEAD)], BF16, tag="strip",
                        name=f"strip{i}h{h}",
                    )
                    for h in range(2)
                ]
                for i in range(n_slabs)
            ]

            def half_transpose(c, qlo):
                """xbar-load rows qlo, qlo+1 of slab c into their half tile."""
                SLABT = slabs[c] + HEAD
                dst = strips[c][qlo // 2][:, : 2 * SLABT].rearrange(
                    "p (q t) -> p q t", q=2
                )
                src = x_ap[starts[c] : starts[c] + SLABT, qlo : qlo + 2, :]
                nc.sync.dma_start_transpose(dst, src)

            n_add = 0
            for c, chunk in enumerate(slabs):
                SLABT = chunk + HEAD
                half_transpose(c, 0)
                half_transpose(c, 2)
                for q in range(Q):
                    half = strips[c][q // 2]
                    base = (q % 2) * SLABT + HEAD
                    oT_full = otp.tile([P, maxc], BF16, tag="oT")
                    oT = oT_full[:, :chunk]
                    for s in range(chunk // SW):
                        pacc = paccp.tile([P, SW], FP32, tag="pacc")
                        for k in range(KTAPS):
                            off = base + s * SW - DIL * k
                            nc.tensor.matmul(
                                pacc[:],
                                w_sb[:, k * P : (k + 1) * P],
                                half[:, off : off + SW],
                                start=(k == 0),
                                stop=(k == KTAPS - 1),
                            )
                        # bias during PSUM->SBUF copy, DVE/ACT alternating
                        dst = oT[:, s * SW : (s + 1) * SW]
                        if n_add % 2 == 0:
                            nc.vector.tensor_scalar_add(dst, pacc[:], bias_sb[:])
                        else:
                            nc.scalar.add(dst, pacc[:], bias_sb[:])
                        n_add += 1
                    # store this row's [i, t] window; SWDGE on the idle
                    # GpSimd queue so stores never block the xbar stream
                    nc.gpsimd.dma_start(
                        o_ap[q, :, starts[c] : starts[c] + chunk], oT
                    )
    nc.compile()
    return nc


_cache = {}
_lock = threading.Lock()


def _get_nc():
    with _lock:
        if "nc" not in _cache:
            _cache["nc"] = build()
        return _cache["nc"]


def prep_inputs(x, weight, bias):
    # w_all[j, k*128 + i] = weight[i, j, k]
    w_all = np.ascontiguousarray(
        np.transpose(np.asarray(weight, np.float32), (1, 2, 0))
        .reshape(P, KTAPS * P)
        .astype(ml_dtypes.bfloat16)
    )
    b2 = np.ascontiguousarray(np.asarray(bias, np.float32).reshape(P, 1))
    xb = np.asarray(x, np.float32).astype(ml_dtypes.bfloat16)
    # per core: x4[tt, q, j] = x[b_q, tt-HEAD, j], 16 zero rows at the top
    # (q-major: the xbar fills transposed rows partition-first, so row
    # r = q*128+j lands at partition j, sub-slab q)
    xi = np.zeros((NCORES, HEAD + T_FULL, B_CORE, P), dtype=ml_dtypes.bfloat16)
    xg = xb.reshape(NCORES, B_CORE, T_FULL, P)
    xi[:, HEAD:, :, :] = np.swapaxes(xg, 1, 2)
    return xi, w_all, b2


def kernel(x, weight, bias, _trace=False):
    xi, w_all, b2 = prep_inputs(x, weight, bias)
    nc = _get_nc()
    in_maps = [
        {"x": np.ascontiguousarray(xi[c]), "w": w_all, "b": b2}
        for c in range(NCORES)
    ]
    res = run_bass_kernel_spmd(nc, in_maps, core_ids=list(range(NCORES)), trace=_trace)
    # device output is [Q, 128(i), T]; restore [Q, T, 128] on host
    out = np.concatenate(
        [
            np.swapaxes(np.asarray(r["o"]), 1, 2).astype(np.float32)
            for r in res.results
        ],
        axis=0,
    )
    if _trace:
        kernel.last_results = res
    return out


# revision 29
# speedup vs baseline: 1.1657x; 1.1657x over previous
"""Causal dilated conv1d (K=3, dilation=2, N=128 channels) on Trainium2.

out[b,t,i] = sum_{j,k} x[b, t-2k, j] * weight[i,j,k] + bias[i]

Strategy (8-core SPMD, pure data parallel over batch; bf16 datapath):
  - each core handles 4 of the 32 batch rows; weight/bias replicated.
  - host interleaves the core's 4 rows with a 16-row zero head:
    x4[tt, q, j] = x[b_q, tt-16, j]  (zeros for tt<16).  DMA xbar
    transposes load each slab directly as [128(j), q, t] strips in SBUF
    (transposed row r = q*128+j lands partition-first), so one transfer
    feeds multiple rows' strips, the PE does no transpose-in, and the
    zero head doubles as causal pad (no edge special-casing).  Each
    slab's transpose is split by row-pair across the two HWDGE queues
    (sync: q2/q3, act: q0/q1) so the two xbar streams run concurrently.
  - per-slab strip tiles (fresh pool tile per slab) keep the slab
    transposes free of false WAR deps: the tile framework tracks
    hazards at tile granularity, not byte ranges.
  - taps: 3 accumulated bf16 matmuls per 512-wide PSUM window, moving
    operand = strip shifted by 2k columns.  This is the ONLY PE work.
  - bias is added during the PSUM->SBUF copy, alternating between the
    DVE (tensor_scalar_add) and ACT (scalar.add) so neither engine is
    the bottleneck and ACT has room to host its xbar half-transposes.
  - the result stays in [i, t] layout: it is stored transposed to DRAM
    ([Q, 128, T], 128 contiguous descriptors per store) and the HOST
    restores [Q, T, 128] during the gather.  This removes the entire
    on-device transpose-out (PE identity matmuls + PSUM + DVE copies).
  - variable slab sizes (1024, 2048x3, 1024): small first slab starts
    the PE earlier, small last slab drains the tail faster.
  - output is bf16; host upconverts to fp32.
"""

import threading

import numpy as np
import ml_dtypes

import concourse.bass as bass  # noqa: F401  (bass types used via bacc/tile)
import concourse.mybir as mybir
import concourse.tile as tile
from concourse import bacc
from concourse.bass_utils import run_bass_kernel_spmd

P = 128
KTAPS = 3
DIL = 2
NCORES = 8
B_FULL, T_FULL = 32, 8192
B_CORE = B_FULL // NCORES  # 4
HEAD = 16  # zero rows prepended on host (causal pad + keeps slabs aligned)

FP32 = mybir.dt.float32
BF16 = mybir.dt.bfloat16


def build(T=T_FULL, slabs=(1024, 2048, 2048, 2048, 1024)):
    """Build the per-core Bass module. Same NEFF runs SPMD on all 8 cores."""
    assert sum(slabs) == T
    nc = bacc.Bacc(
        "TRN2",
        target_bir_lowering=False,
        debug=False,
        enable_asserts=False,
        num_devices=NCORES,
    )
    Q = B_CORE  # interleaved batch rows per core
    x_d = nc.dram_tensor("x", [HEAD + T, Q, P], BF16, kind="ExternalInput")
    w_d = nc.dram_tensor("w", [P, KTAPS * P], BF16, kind="ExternalInput")
    b_d = nc.dram_tensor("b", [P, 1], FP32, kind="ExternalInput")
    # output stays transposed on device: o[q, i, t]; host restores [q, t, i]
    o_d = nc.dram_tensor("o", [Q, P, T], BF16, kind="ExternalOutput")

    x_ap, o_ap = x_d.ap(), o_d.ap()
    SW = 512  # tap-matmul moving width (1 PSUM bank of fp32)
    n_slabs = len(slabs)
    starts = [sum(slabs[:i]) for i in range(n_slabs)]
    maxc = max(slabs)

    with tile.TileContext(nc) as tc:
        with (
            tc.tile_pool(name="const", bufs=1) as cp,
            tc.tile_pool(name="strip", bufs=n_slabs) as sp,
            tc.tile_pool(name="oT", bufs=6) as otp,
            tc.tile_pool(name="pacc", bufs=6, space="PSUM") as paccp,
        ):
            w_sb = cp.tile([P, KTAPS * P], BF16)
            nc.sync.dma_start(w_sb[:], w_d.ap())
            bias_sb = cp.tile([P, 1], FP32)
            nc.sync.dma_start(bias_sb[:], b_d.ap())

            # one strip tile per slab: independent tiles, so the xbar
            # transposes stream back-to-back on the sync queue with no
            # false WAR deps.  (Two CONCURRENT xbar streams on sync+act
            # corrupt data — the xbar is a single shared unit — and >5
            # in-flight transposes stall the sequencer, so: one full-slab
            # transpose per slab, all on sync.)
            strips = [
                sp.tile([P, Q * (maxc + HEAD)], BF16, tag="strip", name=f"strip{i}")
                for i in range(n_slabs)
            ]

            n_add = 0
            for c, chunk in enumerate(slabs):
                SLABT = chunk + HEAD
                nc.sync.dma_start_transpose(
                    strips[c][:, : Q * SLABT].rearrange("p (q t) -> p q t", q=Q),
                    x_ap[starts[c] : starts[c] + SLABT, :, :],
                )
                for q in range(Q):
                    half = strips[c]
                    base = q * SLABT + HEAD
                    oT_full = otp.tile([P, maxc], BF16, tag="oT")
                    oT = oT_full[:, :chunk]
                    for s in range(chunk // SW):
                        pacc = paccp.tile([P, SW], FP32, tag="pacc")
                        for k in range(KTAPS):
                            off = base + s * SW - DIL * k
                            nc.tensor.matmul(
                                pacc[:],
                                w_sb[:, k * P : (k + 1) * P],
                                half[:, off : off + SW],
                                start=(k == 0),
                                stop=(k == KTAPS - 1),
                            )
                        # bias during PSUM->SBUF copy, DVE/ACT alternating
                        dst = oT[:, s * SW : (s + 1) * SW]
                        if n_add % 2 == 0:
                            nc.vector.tensor_scalar_add(dst, pacc[:], bias_sb[:])
                        else:
                            nc.scalar.add(dst, pacc[:], bias_sb[:])
                        n_add += 1
                    # store this row's [i, t] window; SWDGE on the idle
                    # GpSimd queue so stores never block the xbar stream
                    nc.gpsimd.dma_start(
                        o_ap[q, :, starts[c] : starts[c] + chunk], oT
                    )
    nc.compile()
    return nc


_cache = {}
_lock = threading.Lock()


def _get_nc():
    with _lock:
        if "nc" not in _cache:
            _cache["nc"] = build()
        return _cache["nc"]


def prep_inputs(x, weight, bias):
    # w_all[j, k*128 + i] = weight[i, j, k]
    w_all = np.ascontiguousarray(
        np.transpose(np.asarray(weight, np.float32), (1, 2, 0))
        .reshape(P, KTAPS * P)
        .astype(ml_dtypes.bfloat16)
    )
    b2 = np.ascontiguousarray(np.asarray(bias, np.float32).reshape(P, 1))
    xb = np.asarray(x, np.float32).astype(ml_dtypes.bfloat16)
    # per core: x4[tt, q, j] = x[b_q, tt-HEAD, j], 16 zero rows at the top
    # (q-major: the xbar fills transposed rows partition-first, so row
    # r = q*128+j lands at partition j, sub-slab q)
    xi = np.zeros((NCORES, HEAD + T_FULL, B_CORE, P), dtype=ml_dtypes.bfloat16)
    xg = xb.reshape(NCORES, B_CORE, T_FULL, P)
    xi[:, HEAD:, :, :] = np.swapaxes(xg, 1, 2)
    return xi, w_all, b2


def kernel(x, weight, bias, _trace=False):
    xi, w_all, b2 = prep_inputs(x, weight, bias)
    nc = _get_nc()
    in_maps = [
        {"x": np.ascontiguousarray(xi[c]), "w": w_all, "b": b2}
        for c in range(NCORES)
    ]
    res = run_bass_kernel_spmd(nc, in_maps, core_ids=list(range(NCORES)), trace=_trace)
    # device output is [Q, 128(i), T]; restore [Q, T, 128] on host
    out = np.concatenate(
        [
            np.swapaxes(np.asarray(r["o"]), 1, 2).astype(np.float32)
            for r in res.results
        ],
        axis=0,
    )
    if _trace:
        kernel.last_results = res
    return out
